# revision 11
# baseline (speedup 1.0000x reference)
"""AttentiveStatsPool Trainium2 Bass kernel.

Full-input contract: kernel(**inputs) takes the unsharded numpy inputs and
returns the full (B, 2C, 1) output.  Internally shards the batch (B=16)
across 8 NeuronCores (2 samples per core), weights replicated, no cross-core
communication.

Math per sample (mask is all-ones per the problem spec):
  S1 = sum_t x, S2 = sum_t x^2                        (per channel)
  mean0 = S1/T, std0 = sqrt(max(S2/T - mean0^2, 1e-5))
  m1 = w1[:, :C] @ x            (H, T)
  cH = w1[:, C:2C] @ mean0 + w1[:, 2C:] @ std0 + b1   (H,)
  r = relu(m1 + cH)
  LN over H: h = tanh(g1 * (r - mu)*rsqrt(var+1e-5) + be1)
  z = w2 @ h                    (b2 shifts z per channel and softmax over T is
                                 shift-invariant per channel, so b2 drops out)
  u = exp(z), Z = sum_t u, M1 = sum_t u*x, M2 = sum_t u*x^2
  mean = M1/Z, std = sqrt(max(M2/Z - mean^2, 1e-5))
  out = LayerNorm_{3072}(concat(mean, std)) * g2 + be2
"""

import numpy as np
import ml_dtypes

B, C, T, H = 16, 1536, 2000, 128
NCORES = 8
BLOC = B // NCORES          # 2 samples per core
KC = C // 128               # 12 channel chunks
TH = T // 2                 # 1000, half-T psum region
TQ = T // 4                 # 500, one psum bank of f32
EPS = 1e-5

_compiled = {}


# ---------------------------------------------------------------------------
# Workaround for walrus codegen 'Too many sync wait commands': this container's
# walrus supports only ONE sync-wait slot per instruction, but Tile's wait
# assignment can attach several.  Post-pass: move excess waits onto standalone
# InstNoOp carriers spliced immediately before the instruction on the same
# engine (same-engine program order makes this equivalent).
# ---------------------------------------------------------------------------

def _apply_tile_patch():
    import concourse.mybir as mybir
    import concourse.tile as tile
    from concourse.vector_clock import ScopedClock

    if getattr(tile.TileContext, "_wait_split_patched", False):
        return

    MAX_WAITS = 1

    def split_excess_waits(nc):
        for fn in nc.m.functions:
            for bb in fn.blocks:
                il = bb.instructions
                out = []
                changed = False
                for inst in il:
                    si = getattr(inst, "sync_info", None)
                    waits = list(si.on_wait) if si is not None else []
                    if len(waits) > MAX_WAITS:
                        for j, w in enumerate(waits[MAX_WAITS:]):
                            nop = mybir.InstNoOp(
                                name=f"{inst.name}-wsplit{j}",
                                sync_info=mybir.SyncInfo(on_wait=[w], on_update=[]),
                                bass_nofuse=True,
                                engine=inst.engine,
                            )
                            nc.register_instruction(nop, overwrite=True)
                            out.append(nop)
                        si.on_wait = waits[:MAX_WAITS]
                        changed = True
                    out.append(inst)
                if changed:
                    bb.instructions = out

    def _patched_drain_and_barrier(self, tick_clock, wait_clock):
        nc = self.nc
        drain_inst = nc.sync.drain()
        wait_clock.add_sem_waits(
            drain_inst.ins, ScopedClock({None: tick_clock.global_clock})
        )
        nc.all_engine_barrier()
        assert self.sems is not None
        popped = nc._tile_sem_poison_stack.pop()
        assert popped is self._sem_poison
        nc.clear_and_free_semaphores(list(self.sems.allocated().values()))
        nc.all_engine_barrier()
        split_excess_waits(nc)

    tile.TileContext._drain_and_barrier = _patched_drain_and_barrier
    tile.TileContext._wait_split_patched = True


# ---------------------------------------------------------------------------
# Device kernel builder (one NeuronCore, BLOC samples)
# ---------------------------------------------------------------------------

DEBUG = False


def _build():
    import concourse.bass as bass
    import concourse.tile as tile
    import concourse.mybir as mybir
    from contextlib import ExitStack

    _apply_tile_patch()

    f32 = mybir.dt.float32
    bf16 = mybir.dt.bfloat16
    AL = mybir.AluOpType
    AF = mybir.ActivationFunctionType

    nc = bass.Bass(name="attnpool")

    xd = nc.dram_tensor("x", [BLOC, KC, 128, T], f32, kind="ExternalInput")
    wad = nc.dram_tensor("wa", [KC, 128, 128], bf16, kind="ExternalInput")
    wbcd = nc.dram_tensor("wbc", [2 * KC, 128, 128], f32, kind="ExternalInput")
    w2td = nc.dram_tensor("w2t", [KC, 128, 128], bf16, kind="ExternalInput")
    onesd = nc.dram_tensor("ones_bf", [128, 128], bf16, kind="ExternalInput")
    onesfd = nc.dram_tensor("ones_f", [128, 128], f32, kind="ExternalInput")
    b1d = nc.dram_tensor("b1v", [128, 1], f32, kind="ExternalInput")
    g1d = nc.dram_tensor("g1v", [128, 1], f32, kind="ExternalInput")
    be1d = nc.dram_tensor("be1v", [128, 1], f32, kind="ExternalInput")
    g2d = nc.dram_tensor("g2v", [128, 2 * KC], f32, kind="ExternalInput")
    be2d = nc.dram_tensor("be2v", [128, 2 * KC], f32, kind="ExternalInput")
    yd = nc.dram_tensor("y", [BLOC, 2 * KC, 128], f32, kind="ExternalOutput")
    dbg = {}
    if DEBUG:
        for nm, shp in [("dS1", [128, 24]), ("dS2", [128, 24]), ("dmean0", [128, 24]),
                        ("dstd0", [128, 24]), ("dbias", [128, 2]), ("dh", [128, 2, 2000]),
                        ("dZ", [128, 24]), ("dM1", [128, 24]), ("dM2", [128, 24]),
                        ("dr", [128, 2, 2000])]:
            dbg[nm] = nc.dram_tensor(nm, shp, f32, kind="ExternalOutput")

    NB = BLOC * KC  # 24 accum columns, col = b*KC + k

    with tile.TileContext(nc) as tc, ExitStack() as ctx:
        singles = ctx.enter_context(tc.tile_pool(name="singles", bufs=1))
        xpool = ctx.enter_context(tc.tile_pool(name="xcache", bufs=1))
        work = ctx.enter_context(tc.tile_pool(name="work", bufs=1))
        dscr = ctx.enter_context(tc.tile_pool(name="dscr", bufs=2))

        # ---- weights / constants to SBUF ----
        wa_sb = singles.tile([128, KC, 128], bf16)
        nc.sync.dma_start(out=wa_sb, in_=wad.rearrange("k c h -> c k h"))
        wbc_sb = singles.tile([128, 2 * KC, 128], f32)
        nc.sync.dma_start(out=wbc_sb, in_=wbcd.rearrange("k c h -> c k h"))
        w2t_sb = singles.tile([128, KC, 128], bf16)
        nc.sync.dma_start(out=w2t_sb, in_=w2td.rearrange("k h c -> h k c"))
        ones_sb = singles.tile([128, 128], bf16)
        nc.sync.dma_start(out=ones_sb, in_=onesd[:, :])
        onesf_sb = singles.tile([128, 128], f32)
        nc.sync.dma_start(out=onesf_sb, in_=onesfd[:, :])
        b1_sb = singles.tile([128, 1], f32)
        nc.sync.dma_start(out=b1_sb, in_=b1d[:, :])
        g1_sb = singles.tile([128, 1], f32)
        nc.sync.dma_start(out=g1_sb, in_=g1d[:, :])
        be1_sb = singles.tile([128, 1], f32)
        nc.sync.dma_start(out=be1_sb, in_=be1d[:, :])
        g2_sb = singles.tile([128, 2 * KC], f32)
        nc.sync.dma_start(out=g2_sb, in_=g2d[:, :])
        be2_sb = singles.tile([128, 2 * KC], f32)
        nc.sync.dma_start(out=be2_sb, in_=be2d[:, :])

        eps_sb = singles.tile([128, 1], f32)
        nc.vector.memset(eps_sb, EPS)

        # ---- persistent SBUF state ----
        x_bf = xpool.tile([128, BLOC, KC, T], bf16)          # 96 KB/part
        r_raw = work.tile([128, BLOC, T], bf16)              # m1 -> r -> t1 -> t3 -> h
        accS1 = work.tile([128, NB], f32)
        accS2 = work.tile([128, NB], f32)
        accZ = work.tile([128, NB], f32)
        accM1 = work.tile([128, NB], f32)
        accM2 = work.tile([128, NB], f32)
        mean0 = work.tile([128, NB], f32)
        std0 = work.tile([128, NB], f32)
        biasv = work.tile([128, BLOC], f32)
        mu_bf = work.tile([128, T], bf16)
        v0_f = work.tile([128, T], f32)
        rs_f = work.tile([128, T], f32)
        r2_bf = work.tile([128, T], bf16)

        # ================= pass 1: load x, m1 matmuls, S1/S2 =================
        # PSUM matmul outputs must stay inside one 2KB bank: one [128, 512]
        # f32 tile (= exactly one bank) per (sample, T-quarter), 500 used.
        with tc.tile_pool(name="pm1", bufs=1, space="PSUM") as pm1:
            m1ps = {}
            for b in range(BLOC):
                for q in range(4):
                    m1ps[(b, q)] = pm1.tile(
                        [128, 512], f32, tag=f"m1_{b}_{q}", name=f"m1_{b}_{q}"
                    )

            for k in range(KC):
                for b in range(BLOC):
                    nc.gpsimd.dma_start(out=x_bf[:, b, k, :], in_=xd[b, k, :, :])
                    for q in range(4):
                        nc.tensor.matmul(
                            m1ps[(b, q)][:, 0:TQ],
                            wa_sb[:, k, :],
                            x_bf[:, b, k, q * TQ:(q + 1) * TQ],
                            start=(k == 0),
                            stop=(k == KC - 1),
                        )
                    col = b * KC + k
                    s1scr = dscr.tile([128, T], bf16, tag="dscr")
                    nc.vector.tensor_scalar(
                        out=s1scr, in0=x_bf[:, b, k, :], scalar1=1.0, scalar2=0.0,
                        op0=AL.mult, op1=AL.add, accum_out=accS1[:, col:col + 1],
                    )
                    s2scr = dscr.tile([128, T], bf16, tag="dscr")
                    nc.vector.scalar_tensor_tensor(
                        out=s2scr, in0=x_bf[:, b, k, :], scalar=1.0,
                        in1=x_bf[:, b, k, :], op0=AL.mult, op1=AL.mult,
                        accum_out=accS2[:, col:col + 1],
                    )

            # evacuate raw m1 to SBUF (bf16)
            for b in range(BLOC):
                for q in range(4):
                    nc.scalar.copy(
                        out=r_raw[:, b, q * TQ:(q + 1) * TQ],
                        in_=m1ps[(b, q)][:, 0:TQ],
                    )

        # ---- mean0 / std0 ----
        nc.vector.tensor_scalar_mul(out=mean0, in0=accS1, scalar1=1.0 / T)
        nc.vector.tensor_scalar_mul(out=std0, in0=accS2, scalar1=1.0 / T)
        m0sq = work.tile([128, NB], f32)
        nc.vector.scalar_tensor_tensor(
            out=m0sq, in0=mean0, scalar=1.0, in1=mean0, op0=AL.mult, op1=AL.mult
        )
        nc.vector.tensor_sub(out=std0, in0=std0, in1=m0sq)
        nc.vector.tensor_scalar_max(out=std0, in0=std0, scalar1=EPS)
        nc.scalar.sqrt(out=std0, in_=std0)

        # ---- cH = w1b @ mean0 + w1c @ std0 (+ b1) ----
        with tc.tile_pool(name="pmid", bufs=1, space="PSUM") as pmid:
            for b in range(BLOC):
                chps = pmid.tile([128, 1], f32, tag=f"ch{b}")
                for j in range(2 * KC):
                    k = j % KC
                    src = mean0 if j < KC else std0
                    nc.tensor.matmul(
                        chps,
                        wbc_sb[:, j, :],
                        src[:, b * KC + k:b * KC + k + 1],
                        start=(j == 0),
                        stop=(j == 2 * KC - 1),
                    )
                nc.vector.tensor_add(out=biasv[:, b:b + 1], in0=chps, in1=b1_sb)

        # ================= LN over H per sample =================
        with tc.tile_pool(name="pstats", bufs=1, space="PSUM") as pst:
            for b in range(BLOC):
                # r = relu(m1 + cH)  (in place on r_raw)
                nc.scalar.activation(
                    out=r_raw[:, b, :], in_=r_raw[:, b, :], func=AF.Relu,
                    bias=biasv[:, b:b + 1], scale=1.0,
                )
                if DEBUG:
                    rdump = work.tile([128, T], f32, tag="rdump")
                    nc.vector.tensor_copy(out=rdump, in_=r_raw[:, b, :])
                    nc.sync.dma_start(out=dbg["dr"][:, b, :], in_=rdump)
                nc.scalar.square(out=r2_bf, in_=r_raw[:, b, :])
                # [128, 4, 512] = 4 banks; matmul q writes [:, q, 0:500]
                # (bank-aligned).  Downstream ops use the [:, :, 0:500] view.
                s1b = pst.tile([128, 4, 512], f32, tag="s1b")
                s2b = pst.tile([128, 4, 512], f32, tag="s2b")
                for q in range(4):
                    sl = slice(q * TQ, (q + 1) * TQ)
                    nc.tensor.matmul(
                        s1b[:, q, 0:TQ], ones_sb, r_raw[:, b, sl],
                        start=True, stop=True,
                    )
                    nc.tensor.matmul(
                        s2b[:, q, 0:TQ], ones_sb, r2_bf[:, sl],
                        start=True, stop=True,
                    )
                s1v = s1b[:, :, 0:TQ]
                s2v = s2b[:, :, 0:TQ]
                muv = mu_bf.rearrange("p (q t) -> p q t", q=4)
                v0v = v0_f.rearrange("p (q t) -> p q t", q=4)
                rsv = rs_f.rearrange("p (q t) -> p q t", q=4)
                # mu = s1b/H ; var = s2b/H - mu^2 ; rs = 1/sqrt(var+eps)
                nc.scalar.mul(out=muv, in_=s1v, mul=1.0 / H)
                nc.vector.scalar_tensor_tensor(
                    out=v0v, in0=muv, scalar=1.0, in1=muv,
                    op0=AL.mult, op1=AL.mult,
                )
                nc.vector.scalar_tensor_tensor(
                    out=v0v, in0=s2v, scalar=1.0 / H, in1=v0v,
                    op0=AL.mult, op1=AL.subtract,
                )
                # rs = 1/sqrt(var+eps) = exp(-0.5*ln(var+eps)); custom-DVE
                # reciprocal ops are unavailable (InstISA rejected by this
                # walrus) and exact reciprocal is 8 cyc/elem.  Clamp var at 0:
                # rounding can push the E[r^2]-mu^2 form slightly negative.
                nc.vector.tensor_scalar_max(out=v0_f, in0=v0_f, scalar1=0.0)
                nc.scalar.activation(
                    out=rs_f, in_=v0_f, func=AF.Ln, bias=eps_sb, scale=1.0
                )
                nc.scalar.activation(
                    out=v0_f, in_=rs_f, func=AF.Exp, scale=-0.5
                )
                # h = tanh(g1*(r-mu)*rs + be1)   (in place on r_raw)
                nc.vector.tensor_sub(
                    out=r_raw[:, b, :], in0=r_raw[:, b, :], in1=mu_bf
                )
                nc.vector.tensor_mul(
                    out=r_raw[:, b, :], in0=r_raw[:, b, :], in1=v0_f
                )
                nc.scalar.activation(
                    out=r_raw[:, b, :], in_=r_raw[:, b, :], func=AF.Tanh,
                    bias=be1_sb, scale=g1_sb,
                )
                if DEBUG:
                    hdump = work.tile([128, T], f32, tag="hdump")
                    nc.vector.tensor_copy(out=hdump, in_=r_raw[:, b, :])
                    nc.sync.dma_start(out=dbg["dh"][:, b, :], in_=hdump)

        # ================= pass 2: z, exp, weighted sums =================
        with tc.tile_pool(name="pz", bufs=2, space="PSUM") as pz:
            for b in range(BLOC):
                for k in range(KC):
                    zps = pz.tile([128, 4, 512], f32, tag="z")
                    for q in range(4):
                        nc.tensor.matmul(
                            zps[:, q, 0:TQ], w2t_sb[:, k, :],
                            r_raw[:, b, q * TQ:(q + 1) * TQ],
                            start=True, stop=True,
                        )
                    col = b * KC + k
                    u_bf = dscr.tile([128, T], bf16, tag="u")
                    nc.scalar.activation(
                        out=u_bf.rearrange("p (q t) -> p q t", q=4),
                        in_=zps[:, :, 0:TQ], func=AF.Exp,
                        accum_out=accZ[:, col:col + 1],
                    )
                    p_bf = dscr.tile([128, T], bf16, tag="p")
                    nc.vector.scalar_tensor_tensor(
                        out=p_bf, in0=u_bf, scalar=1.0, in1=x_bf[:, b, k, :],
                        op0=AL.mult, op1=AL.mult,
                        accum_out=accM1[:, col:col + 1],
                    )
                    q_bf = dscr.tile([128, T], bf16, tag="q")
                    nc.vector.scalar_tensor_tensor(
                        out=q_bf, in0=p_bf, scalar=1.0, in1=x_bf[:, b, k, :],
                        op0=AL.mult, op1=AL.mult,
                        accum_out=accM2[:, col:col + 1],
                    )

        # ================= final stats + LayerNorm(3072) =================
        if DEBUG:
            nc.sync.dma_start(out=dbg["dS1"][:, :], in_=accS1)
            nc.sync.dma_start(out=dbg["dS2"][:, :], in_=accS2)
            nc.sync.dma_start(out=dbg["dmean0"][:, :], in_=mean0)
            nc.sync.dma_start(out=dbg["dstd0"][:, :], in_=std0)
            nc.sync.dma_start(out=dbg["dbias"][:, :], in_=biasv)
            nc.sync.dma_start(out=dbg["dZ"][:, :], in_=accZ)
            nc.sync.dma_start(out=dbg["dM1"][:, :], in_=accM1)
            nc.sync.dma_start(out=dbg["dM2"][:, :], in_=accM2)
        zr = work.tile([128, NB], f32)
        nc.vector.reciprocal(out=zr, in_=accZ)
        vmean = work.tile([128, NB], f32)
        nc.vector.tensor_mul(out=vmean, in0=accM1, in1=zr)
        ve2 = work.tile([128, NB], f32)
        nc.vector.tensor_mul(out=ve2, in0=accM2, in1=zr)
        vmsq = work.tile([128, NB], f32)
        nc.vector.scalar_tensor_tensor(
            out=vmsq, in0=vmean, scalar=1.0, in1=vmean, op0=AL.mult, op1=AL.mult
        )
        nc.vector.tensor_sub(out=ve2, in0=ve2, in1=vmsq)
        nc.vector.tensor_scalar_max(out=ve2, in0=ve2, scalar1=EPS)
        nc.scalar.sqrt(out=ve2, in_=ve2)  # ve2 = std (128, NB)

        with tc.tile_pool(name="pfin", bufs=1, space="PSUM") as pf:
            for b in range(BLOC):
                v = work.tile([128, 2 * KC], f32, tag="vfin")
                nc.vector.tensor_copy(out=v[:, 0:KC], in_=vmean[:, b * KC:(b + 1) * KC])
                nc.vector.tensor_copy(out=v[:, KC:2 * KC], in_=ve2[:, b * KC:(b + 1) * KC])
                v2 = work.tile([128, 2 * KC], f32, tag="v2fin")
                nc.scalar.square(out=v2, in_=v)
                svp = pf.tile([128, 2 * KC], f32, tag="sv")
                nc.tensor.matmul(svp, onesf_sb, v, start=True, stop=True)
                sv2p = pf.tile([128, 2 * KC], f32, tag="sv2")
                nc.tensor.matmul(sv2p, onesf_sb, v2, start=True, stop=True)
                muf = work.tile([128, 1], f32, tag="muf")
                nc.vector.tensor_reduce(
                    out=muf, in_=svp, axis=mybir.AxisListType.X, op=AL.add
                )
                s2r = work.tile([128, 1], f32, tag="s2r")
                nc.vector.tensor_reduce(
                    out=s2r, in_=sv2p, axis=mybir.AxisListType.X, op=AL.add
                )
                nc.vector.tensor_scalar_mul(out=muf, in0=muf, scalar1=1.0 / (2 * C))
                musq = work.tile([128, 1], f32, tag="musq")
                nc.vector.scalar_tensor_tensor(
                    out=musq, in0=muf, scalar=1.0, in1=muf, op0=AL.mult, op1=AL.mult
                )
                nc.vector.scalar_tensor_tensor(
                    out=s2r, in0=s2r, scalar=1.0 / (2 * C), in1=musq,
                    op0=AL.mult, op1=AL.subtract,
                )
                nc.scalar.activation(
                    out=s2r, in_=s2r, func=AF.Sqrt, bias=eps_sb, scale=1.0
                )
                nc.vector.reciprocal(out=s2r, in_=s2r)
                vout = work.tile([128, 2 * KC], f32, tag="vout")
                nc.vector.tensor_scalar(
                    out=vout, in0=v, scalar1=muf, scalar2=s2r,
                    op0=AL.subtract, op1=AL.mult,
                )
                nc.vector.tensor_mul(out=vout, in0=vout, in1=g2_sb)
                nc.vector.tensor_add(out=vout, in0=vout, in1=be2_sb)
                nc.sync.dma_start(
                    out=yd[b].rearrange("k p -> p k"), in_=vout
                )

    return nc


def _get_nc():
    if "nc" not in _compiled:
        _compiled["nc"] = _build()
    return _compiled["nc"]


def kernel(x, mask, w1, b1, g1, be1, w2, b2, g2, be2, _trace=False, _tmpdir=None):
    from concourse.bass_utils import run_bass_kernel_spmd

    bf = ml_dtypes.bfloat16
    x = np.asarray(x, dtype=np.float32)

    w1a = np.ascontiguousarray(w1[:, :C].T).reshape(KC, 128, H).astype(bf)
    w1bT = np.ascontiguousarray(w1[:, C:2 * C].T).reshape(KC, 128, H).astype(np.float32)
    w1cT = np.ascontiguousarray(w1[:, 2 * C:].T).reshape(KC, 128, H).astype(np.float32)
    wbc = np.concatenate([w1bT, w1cT], axis=0)
    w2t = np.ascontiguousarray(
        np.asarray(w2, np.float32).reshape(KC, 128, H).transpose(0, 2, 1)
    ).astype(bf)

    common = {
        "wa": w1a,
        "wbc": wbc,
        "w2t": w2t,
        "ones_bf": np.ones((128, 128), dtype=bf),
        "ones_f": np.ones((128, 128), dtype=np.float32),
        "b1v": np.asarray(b1, np.float32).reshape(128, 1),
        "g1v": np.asarray(g1, np.float32).reshape(128, 1),
        "be1v": np.asarray(be1, np.float32).reshape(128, 1),
        "g2v": np.ascontiguousarray(np.asarray(g2, np.float32).reshape(2 * KC, 128).T),
        "be2v": np.ascontiguousarray(np.asarray(be2, np.float32).reshape(2 * KC, 128).T),
    }

    in_maps = []
    for i in range(NCORES):
        xi = np.ascontiguousarray(
            x[i * BLOC:(i + 1) * BLOC].reshape(BLOC, KC, 128, T)
        )
        in_maps.append({"x": xi, **common})

    nc = _get_nc()
    kwargs = {}
    if _trace:
        kwargs = {"trace": True, "tmpdir": _tmpdir}
    res = run_bass_kernel_spmd(nc, in_maps, core_ids=list(range(NCORES)), **kwargs)

    out = np.empty((B, 2 * C, 1), dtype=np.float32)
    for i in range(NCORES):
        yi = res.results[i]["y"].reshape(BLOC, 2 * C)
        out[i * BLOC:(i + 1) * BLOC, :, 0] = yi
    if _trace:
        return out, res
    return out


# revision 14
# speedup vs baseline: 1.1438x; 1.1438x over previous
"""AttentiveStatsPool Trainium2 Bass kernel.

Full-input contract: kernel(**inputs) takes the unsharded numpy inputs and
returns the full (B, 2C, 1) output.  Internally shards the batch (B=16)
across 8 NeuronCores (2 samples per core), weights replicated, no cross-core
communication.

Math per sample (mask is all-ones per the problem spec):
  S1 = sum_t x, S2 = sum_t x^2                        (per channel)
  mean0 = S1/T, std0 = sqrt(max(S2/T - mean0^2, 1e-5))
  m1 = w1[:, :C] @ x            (H, T)
  cH = w1[:, C:2C] @ mean0 + w1[:, 2C:] @ std0 + b1   (H,)
  r = relu(m1 + cH)
  LN over H: h = tanh(g1 * (r - mu)*rsqrt(var+1e-5) + be1)
  z = w2 @ h                    (b2 shifts z per channel and softmax over T is
                                 shift-invariant per channel, so b2 drops out)
  u = exp(z), Z = sum_t u, M1 = sum_t u*x, M2 = sum_t u*x^2
  mean = M1/Z, std = sqrt(max(M2/Z - mean^2, 1e-5))
  out = LayerNorm_{3072}(concat(mean, std)) * g2 + be2
"""

import numpy as np
import ml_dtypes

B, C, T, H = 16, 1536, 2000, 128
NCORES = 8
BLOC = B // NCORES          # 2 samples per core
KC = C // 128               # 12 channel chunks
TH = T // 2                 # 1000, half-T psum region
TQ = T // 4                 # 500, one psum bank of f32
TSPL = 1200                 # S2 split point: [0:TSPL] on ACT, rest on DVE
EPS = 1e-5

_compiled = {}


# ---------------------------------------------------------------------------
# Workaround for walrus codegen 'Too many sync wait commands': this container's
# walrus supports only ONE sync-wait slot per instruction, but Tile's wait
# assignment can attach several.  Post-pass: move excess waits onto standalone
# InstNoOp carriers spliced immediately before the instruction on the same
# engine (same-engine program order makes this equivalent).
# ---------------------------------------------------------------------------

def _apply_tile_patch():
    import concourse.mybir as mybir
    import concourse.tile as tile
    from concourse.vector_clock import ScopedClock

    if getattr(tile.TileContext, "_wait_split_patched", False):
        return

    MAX_WAITS = 1

    def split_excess_waits(nc):
        for fn in nc.m.functions:
            for bb in fn.blocks:
                il = bb.instructions
                out = []
                changed = False
                for inst in il:
                    si = getattr(inst, "sync_info", None)
                    waits = list(si.on_wait) if si is not None else []
                    if len(waits) > MAX_WAITS:
                        for j, w in enumerate(waits[MAX_WAITS:]):
                            nop = mybir.InstNoOp(
                                name=f"{inst.name}-wsplit{j}",
                                sync_info=mybir.SyncInfo(on_wait=[w], on_update=[]),
                                bass_nofuse=True,
                                engine=inst.engine,
                            )
                            nc.register_instruction(nop, overwrite=True)
                            out.append(nop)
                        si.on_wait = waits[:MAX_WAITS]
                        changed = True
                    out.append(inst)
                if changed:
                    bb.instructions = out

    def _patched_drain_and_barrier(self, tick_clock, wait_clock):
        nc = self.nc
        drain_inst = nc.sync.drain()
        wait_clock.add_sem_waits(
            drain_inst.ins, ScopedClock({None: tick_clock.global_clock})
        )
        nc.all_engine_barrier()
        assert self.sems is not None
        popped = nc._tile_sem_poison_stack.pop()
        assert popped is self._sem_poison
        nc.clear_and_free_semaphores(list(self.sems.allocated().values()))
        nc.all_engine_barrier()
        split_excess_waits(nc)

    tile.TileContext._drain_and_barrier = _patched_drain_and_barrier
    tile.TileContext._wait_split_patched = True


# ---------------------------------------------------------------------------
# Device kernel builder (one NeuronCore, BLOC samples)
# ---------------------------------------------------------------------------

DEBUG = False


def _build():
    import concourse.bass as bass
    import concourse.tile as tile
    import concourse.mybir as mybir
    from contextlib import ExitStack

    _apply_tile_patch()

    f32 = mybir.dt.float32
    bf16 = mybir.dt.bfloat16
    AL = mybir.AluOpType
    AF = mybir.ActivationFunctionType

    nc = bass.Bass(name="attnpool")

    xd = nc.dram_tensor("x", [BLOC, KC, 128, T], f32, kind="ExternalInput")
    wad = nc.dram_tensor("wa", [128, KC, 128], bf16, kind="ExternalInput")
    wbcd = nc.dram_tensor("wbc", [128, 2 * KC, 128], f32, kind="ExternalInput")
    w2td = nc.dram_tensor("w2t", [128, KC, 128], bf16, kind="ExternalInput")
    onesd = nc.dram_tensor("ones_bf", [128, 128], bf16, kind="ExternalInput")
    onesfd = nc.dram_tensor("ones_f", [128, 128], f32, kind="ExternalInput")
    b1d = nc.dram_tensor("b1v", [128, 1], f32, kind="ExternalInput")
    g1d = nc.dram_tensor("g1v", [128, 1], f32, kind="ExternalInput")
    be1d = nc.dram_tensor("be1v", [128, 1], f32, kind="ExternalInput")
    g2d = nc.dram_tensor("g2v", [128, 2 * KC], f32, kind="ExternalInput")
    be2d = nc.dram_tensor("be2v", [128, 2 * KC], f32, kind="ExternalInput")
    yd = nc.dram_tensor("y", [BLOC, 128, 2 * KC], f32, kind="ExternalOutput")
    dbg = {}
    if DEBUG:
        for nm, shp in [("dS1", [128, 24]), ("dS2", [128, 24]), ("dmean0", [128, 24]),
                        ("dstd0", [128, 24]), ("dbias", [128, 2]), ("dh", [128, 2, 2000]),
                        ("dZ", [128, 24]), ("dM1", [128, 24]), ("dM2", [128, 24]),
                        ("dr", [128, 2, 2000])]:
            dbg[nm] = nc.dram_tensor(nm, shp, f32, kind="ExternalOutput")

    NB = BLOC * KC  # 24 accum columns, col = b*KC + k

    with tile.TileContext(nc) as tc, ExitStack() as ctx:
        singles = ctx.enter_context(tc.tile_pool(name="singles", bufs=1))
        xpool = ctx.enter_context(tc.tile_pool(name="xcache", bufs=1))
        work = ctx.enter_context(tc.tile_pool(name="work", bufs=1))
        dscr = ctx.enter_context(tc.tile_pool(name="dscr", bufs=2))

        # ---- weights / constants to SBUF ----
        wa_sb = singles.tile([128, KC, 128], bf16)
        nc.sync.dma_start(out=wa_sb, in_=wad[:, :, :])
        wbc_sb = singles.tile([128, 2 * KC, 128], f32)
        nc.sync.dma_start(out=wbc_sb, in_=wbcd[:, :, :])
        w2t_sb = singles.tile([128, KC, 128], bf16)
        nc.sync.dma_start(out=w2t_sb, in_=w2td[:, :, :])
        ones_sb = singles.tile([128, 128], bf16)
        nc.sync.dma_start(out=ones_sb, in_=onesd[:, :])
        onesf_sb = singles.tile([128, 128], f32)
        nc.sync.dma_start(out=onesf_sb, in_=onesfd[:, :])
        b1_sb = singles.tile([128, 1], f32)
        nc.sync.dma_start(out=b1_sb, in_=b1d[:, :])
        g1_sb = singles.tile([128, 1], f32)
        nc.sync.dma_start(out=g1_sb, in_=g1d[:, :])
        be1_sb = singles.tile([128, 1], f32)
        nc.sync.dma_start(out=be1_sb, in_=be1d[:, :])
        g2_sb = singles.tile([128, 2 * KC], f32)
        nc.sync.dma_start(out=g2_sb, in_=g2d[:, :])
        be2_sb = singles.tile([128, 2 * KC], f32)
        nc.sync.dma_start(out=be2_sb, in_=be2d[:, :])

        eps_sb = singles.tile([128, 1], f32)
        nc.vector.memset(eps_sb, EPS)

        # ---- persistent SBUF state ----
        x_bf = xpool.tile([128, BLOC, KC, T], bf16)          # 96 KB/part
        r_raw = work.tile([128, BLOC, T], bf16)              # m1 -> r -> t1 -> t3 -> h
        accS1 = work.tile([128, NB], f32)
        accS2a = work.tile([128, NB], f32)
        accS2b = work.tile([128, NB], f32)
        accZ = work.tile([128, NB], f32)
        accM1 = work.tile([128, NB], f32)
        accM2 = work.tile([128, NB], f32)
        mean0 = work.tile([128, NB], f32)
        std0 = work.tile([128, NB], f32)
        biasv = work.tile([128, BLOC], f32)
        mu_bf = work.tile([128, T], bf16)
        v0_f = work.tile([128, T], f32)
        rs_f = work.tile([128, T], f32)
        r2_bf = work.tile([128, T], bf16)

        # ================= pass 1: load x, m1 matmuls, S1/S2 =================
        # PSUM matmul outputs must stay inside one 2KB bank: one [128, 512]
        # f32 tile (= exactly one bank) per (sample, T-quarter), 500 used.
        with tc.tile_pool(name="pm1", bufs=1, space="PSUM") as pm1:
            m1ps = {}
            for b in range(BLOC):
                for q in range(4):
                    m1ps[(b, q)] = pm1.tile(
                        [128, 512], f32, tag=f"m1_{b}_{q}", name=f"m1_{b}_{q}"
                    )

            for k in range(KC):
                for b in range(BLOC):
                    nc.gpsimd.dma_start(out=x_bf[:, b, k, :], in_=xd[b, k, :, :])
                    for q in range(4):
                        nc.tensor.matmul(
                            m1ps[(b, q)][:, 0:TQ],
                            wa_sb[:, k, :],
                            x_bf[:, b, k, q * TQ:(q + 1) * TQ],
                            start=(k == 0),
                            stop=(k == KC - 1),
                        )
                    col = b * KC + k
                    s1scr = dscr.tile([128, T], bf16, tag="dscr")
                    nc.scalar.activation(
                        out=s1scr, in_=x_bf[:, b, k, :], func=AF.Copy,
                        accum_out=accS1[:, col:col + 1],
                    )
                    s2scr = dscr.tile([128, T], bf16, tag="dscr")
                    nc.scalar.activation(
                        out=s2scr[:, 0:TSPL], in_=x_bf[:, b, k, 0:TSPL],
                        func=AF.Square, accum_out=accS2a[:, col:col + 1],
                    )
                    s2scr2 = dscr.tile([128, T], bf16, tag="dscr")
                    nc.vector.scalar_tensor_tensor(
                        out=s2scr2[:, 0:T - TSPL], in0=x_bf[:, b, k, TSPL:T],
                        scalar=1.0, in1=x_bf[:, b, k, TSPL:T],
                        op0=AL.mult, op1=AL.mult,
                        accum_out=accS2b[:, col:col + 1],
                    )

            # evacuate raw m1 to SBUF (bf16)
            for b in range(BLOC):
                for q in range(4):
                    nc.scalar.copy(
                        out=r_raw[:, b, q * TQ:(q + 1) * TQ],
                        in_=m1ps[(b, q)][:, 0:TQ],
                    )

        # ---- mean0 / std0 ----
        nc.vector.tensor_scalar_mul(out=mean0, in0=accS1, scalar1=1.0 / T)
        nc.vector.tensor_add(out=std0, in0=accS2a, in1=accS2b)
        nc.vector.tensor_scalar_mul(out=std0, in0=std0, scalar1=1.0 / T)
        m0sq = work.tile([128, NB], f32)
        nc.vector.scalar_tensor_tensor(
            out=m0sq, in0=mean0, scalar=1.0, in1=mean0, op0=AL.mult, op1=AL.mult
        )
        nc.vector.tensor_sub(out=std0, in0=std0, in1=m0sq)
        nc.vector.tensor_scalar_max(out=std0, in0=std0, scalar1=EPS)
        nc.scalar.activation(out=std0, in_=std0, func=AF.Ln)
        nc.scalar.activation(out=std0, in_=std0, func=AF.Exp, scale=0.5)

        # ---- cH = w1b @ mean0 + w1c @ std0 (+ b1) ----
        with tc.tile_pool(name="pmid", bufs=1, space="PSUM") as pmid:
            for b in range(BLOC):
                chps = pmid.tile([128, 1], f32, tag=f"ch{b}")
                for j in range(2 * KC):
                    k = j % KC
                    src = mean0 if j < KC else std0
                    nc.tensor.matmul(
                        chps,
                        wbc_sb[:, j, :],
                        src[:, b * KC + k:b * KC + k + 1],
                        start=(j == 0),
                        stop=(j == 2 * KC - 1),
                    )
                nc.vector.tensor_add(out=biasv[:, b:b + 1], in0=chps, in1=b1_sb)

        # ================= LN over H per sample =================
        with tc.tile_pool(name="pstats", bufs=1, space="PSUM") as pst:
            for b in range(BLOC):
                # r = relu(m1 + cH)  (in place on r_raw)
                nc.scalar.activation(
                    out=r_raw[:, b, :], in_=r_raw[:, b, :], func=AF.Relu,
                    bias=biasv[:, b:b + 1], scale=1.0,
                )
                if DEBUG:
                    rdump = work.tile([128, T], f32, tag="rdump")
                    nc.vector.tensor_copy(out=rdump, in_=r_raw[:, b, :])
                    nc.sync.dma_start(out=dbg["dr"][:, b, :], in_=rdump)
                nc.vector.tensor_mul(out=r2_bf, in0=r_raw[:, b, :], in1=r_raw[:, b, :])
                # [128, 4, 512] = 4 banks; matmul q writes [:, q, 0:500]
                # (bank-aligned).  Downstream ops use the [:, :, 0:500] view.
                s1b = pst.tile([128, 4, 512], f32, tag="s1b")
                s2b = pst.tile([128, 4, 512], f32, tag="s2b")
                for q in range(4):
                    sl = slice(q * TQ, (q + 1) * TQ)
                    nc.tensor.matmul(
                        s1b[:, q, 0:TQ], ones_sb, r_raw[:, b, sl],
                        start=True, stop=True,
                    )
                    nc.tensor.matmul(
                        s2b[:, q, 0:TQ], ones_sb, r2_bf[:, sl],
                        start=True, stop=True,
                    )
                s1v = s1b[:, :, 0:TQ]
                s2v = s2b[:, :, 0:TQ]
                muv = mu_bf.rearrange("p (q t) -> p q t", q=4)
                v0v = v0_f.rearrange("p (q t) -> p q t", q=4)
                rsv = rs_f.rearrange("p (q t) -> p q t", q=4)
                # mu = s1b/H ; var = s2b/H - mu^2 ; rs = 1/sqrt(var+eps)
                nc.scalar.mul(out=muv, in_=s1v, mul=1.0 / H)
                nc.vector.scalar_tensor_tensor(
                    out=v0v, in0=muv, scalar=1.0, in1=muv,
                    op0=AL.mult, op1=AL.mult,
                )
                nc.vector.scalar_tensor_tensor(
                    out=v0v, in0=s2v, scalar=1.0 / H, in1=v0v,
                    op0=AL.mult, op1=AL.subtract,
                )
                # rs = 1/sqrt(var+eps) = exp(-0.5*ln(var+eps)); custom-DVE
                # reciprocal ops are unavailable (InstISA rejected by this
                # walrus) and exact reciprocal is 8 cyc/elem.  Clamp var at 0:
                # rounding can push the E[r^2]-mu^2 form slightly negative.
                nc.vector.tensor_scalar_max(out=v0_f, in0=v0_f, scalar1=0.0)
                nc.scalar.activation(
                    out=rs_f, in_=v0_f, func=AF.Ln, bias=eps_sb, scale=1.0
                )
                nc.scalar.activation(
                    out=v0_f, in_=rs_f, func=AF.Exp, scale=-0.5
                )
                # h = tanh(g1*(r-mu)*rs + be1)   (in place on r_raw)
                nc.vector.tensor_sub(
                    out=r_raw[:, b, :], in0=r_raw[:, b, :], in1=mu_bf
                )
                nc.vector.tensor_mul(
                    out=r_raw[:, b, :], in0=r_raw[:, b, :], in1=v0_f
                )
                nc.scalar.activation(
                    out=r_raw[:, b, :], in_=r_raw[:, b, :], func=AF.Tanh,
                    bias=be1_sb, scale=g1_sb,
                )
                if DEBUG:
                    hdump = work.tile([128, T], f32, tag="hdump")
                    nc.vector.tensor_copy(out=hdump, in_=r_raw[:, b, :])
                    nc.sync.dma_start(out=dbg["dh"][:, b, :], in_=hdump)

        # ================= pass 2: z, exp, weighted sums =================
        with tc.tile_pool(name="pz", bufs=2, space="PSUM") as pz:
            for b in range(BLOC):
                for k in range(KC):
                    zps = pz.tile([128, 4, 512], f32, tag="z")
                    for q in range(4):
                        nc.tensor.matmul(
                            zps[:, q, 0:TQ], w2t_sb[:, k, :],
                            r_raw[:, b, q * TQ:(q + 1) * TQ],
                            start=True, stop=True,
                        )
                    col = b * KC + k
                    u_bf = dscr.tile([128, T], bf16, tag="u")
                    nc.scalar.activation(
                        out=u_bf.rearrange("p (q t) -> p q t", q=4),
                        in_=zps[:, :, 0:TQ], func=AF.Exp,
                        accum_out=accZ[:, col:col + 1],
                    )
                    p_bf = dscr.tile([128, T], bf16, tag="p")
                    nc.vector.scalar_tensor_tensor(
                        out=p_bf, in0=u_bf, scalar=1.0, in1=x_bf[:, b, k, :],
                        op0=AL.mult, op1=AL.mult,
                        accum_out=accM1[:, col:col + 1],
                    )
                    q_bf = dscr.tile([128, T], bf16, tag="q")
                    nc.vector.scalar_tensor_tensor(
                        out=q_bf, in0=p_bf, scalar=1.0, in1=x_bf[:, b, k, :],
                        op0=AL.mult, op1=AL.mult,
                        accum_out=accM2[:, col:col + 1],
                    )

        # ================= final stats + LayerNorm(3072) =================
        if DEBUG:
            nc.sync.dma_start(out=dbg["dS1"][:, :], in_=accS1)
            nc.sync.dma_start(out=dbg["dS2"][:, :], in_=accS2)
            nc.sync.dma_start(out=dbg["dmean0"][:, :], in_=mean0)
            nc.sync.dma_start(out=dbg["dstd0"][:, :], in_=std0)
            nc.sync.dma_start(out=dbg["dbias"][:, :], in_=biasv)
            nc.sync.dma_start(out=dbg["dZ"][:, :], in_=accZ)
            nc.sync.dma_start(out=dbg["dM1"][:, :], in_=accM1)
            nc.sync.dma_start(out=dbg["dM2"][:, :], in_=accM2)
        zr = work.tile([128, NB], f32)
        nc.vector.reciprocal(out=zr, in_=accZ)
        vmean = work.tile([128, NB], f32)
        nc.vector.tensor_mul(out=vmean, in0=accM1, in1=zr)
        ve2 = work.tile([128, NB], f32)
        nc.vector.tensor_mul(out=ve2, in0=accM2, in1=zr)
        vmsq = work.tile([128, NB], f32)
        nc.vector.scalar_tensor_tensor(
            out=vmsq, in0=vmean, scalar=1.0, in1=vmean, op0=AL.mult, op1=AL.mult
        )
        nc.vector.tensor_sub(out=ve2, in0=ve2, in1=vmsq)
        nc.vector.tensor_scalar_max(out=ve2, in0=ve2, scalar1=EPS)
        nc.scalar.activation(out=ve2, in_=ve2, func=AF.Ln)
        nc.scalar.activation(out=ve2, in_=ve2, func=AF.Exp, scale=0.5)  # std

        with tc.tile_pool(name="pfin", bufs=1, space="PSUM") as pf:
            for b in range(BLOC):
                v = work.tile([128, 2 * KC], f32, tag="vfin")
                nc.vector.tensor_copy(out=v[:, 0:KC], in_=vmean[:, b * KC:(b + 1) * KC])
                nc.vector.tensor_copy(out=v[:, KC:2 * KC], in_=ve2[:, b * KC:(b + 1) * KC])
                v2 = work.tile([128, 2 * KC], f32, tag="v2fin")
                nc.scalar.square(out=v2, in_=v)
                svp = pf.tile([128, 2 * KC], f32, tag="sv")
                nc.tensor.matmul(svp, onesf_sb, v, start=True, stop=True)
                sv2p = pf.tile([128, 2 * KC], f32, tag="sv2")
                nc.tensor.matmul(sv2p, onesf_sb, v2, start=True, stop=True)
                muf = work.tile([128, 1], f32, tag="muf")
                nc.vector.tensor_reduce(
                    out=muf, in_=svp, axis=mybir.AxisListType.X, op=AL.add
                )
                s2r = work.tile([128, 1], f32, tag="s2r")
                nc.vector.tensor_reduce(
                    out=s2r, in_=sv2p, axis=mybir.AxisListType.X, op=AL.add
                )
                nc.vector.tensor_scalar_mul(out=muf, in0=muf, scalar1=1.0 / (2 * C))
                musq = work.tile([128, 1], f32, tag="musq")
                nc.vector.scalar_tensor_tensor(
                    out=musq, in0=muf, scalar=1.0, in1=muf, op0=AL.mult, op1=AL.mult
                )
                nc.vector.scalar_tensor_tensor(
                    out=s2r, in0=s2r, scalar=1.0 / (2 * C), in1=musq,
                    op0=AL.mult, op1=AL.subtract,
                )
                nc.scalar.activation(
                    out=s2r, in_=s2r, func=AF.Ln, bias=eps_sb, scale=1.0
                )
                nc.scalar.activation(out=s2r, in_=s2r, func=AF.Exp, scale=-0.5)
                vout = work.tile([128, 2 * KC], f32, tag="vout")
                nc.vector.tensor_scalar(
                    out=vout, in0=v, scalar1=muf, scalar2=s2r,
                    op0=AL.subtract, op1=AL.mult,
                )
                nc.vector.tensor_mul(out=vout, in0=vout, in1=g2_sb)
                nc.vector.tensor_add(out=vout, in0=vout, in1=be2_sb)
                nc.sync.dma_start(out=yd[b, :, :], in_=vout)

    return nc


def _get_nc():
    if "nc" not in _compiled:
        _compiled["nc"] = _build()
    return _compiled["nc"]


def _prep_common(w1, b1, g1, be1, w2, g2, be2):
    bf = ml_dtypes.bfloat16
    # SBUF-layout weights (partition-major, contiguous DMA):
    # wa[c, k, h] = w1[h, 128k+c] ; wbc[c, j, h] ; w2t[h, k, c] = w2[128k+c, h]
    w1 = np.asarray(w1, np.float32)
    w1a = np.ascontiguousarray(
        w1[:, :C].T.reshape(KC, 128, H).transpose(1, 0, 2)).astype(bf)
    w1bT = w1[:, C:2 * C].T.reshape(KC, 128, H)
    w1cT = w1[:, 2 * C:].T.reshape(KC, 128, H)
    wbc = np.ascontiguousarray(
        np.concatenate([w1bT, w1cT], axis=0).transpose(1, 0, 2)
    ).astype(np.float32)
    w2t = np.ascontiguousarray(
        np.asarray(w2, np.float32).reshape(KC, 128, H).transpose(2, 0, 1)
    ).astype(bf)

    return {
        "wa": w1a,
        "wbc": wbc,
        "w2t": w2t,
        "ones_bf": np.ones((128, 128), dtype=bf),
        "ones_f": np.ones((128, 128), dtype=np.float32),
        "b1v": np.asarray(b1, np.float32).reshape(128, 1),
        "g1v": np.asarray(g1, np.float32).reshape(128, 1),
        "be1v": np.asarray(be1, np.float32).reshape(128, 1),
        "g2v": np.ascontiguousarray(np.asarray(g2, np.float32).reshape(2 * KC, 128).T),
        "be2v": np.ascontiguousarray(np.asarray(be2, np.float32).reshape(2 * KC, 128).T),
    }


def kernel(x, mask, w1, b1, g1, be1, w2, b2, g2, be2, _trace=False, _tmpdir=None):
    from concourse.bass_utils import run_bass_kernel_spmd

    x = np.asarray(x, dtype=np.float32)
    common = _prep_common(w1, b1, g1, be1, w2, g2, be2)

    in_maps = []
    for i in range(NCORES):
        xi = np.ascontiguousarray(
            x[i * BLOC:(i + 1) * BLOC].reshape(BLOC, KC, 128, T)
        )
        in_maps.append({"x": xi, **common})

    nc = _get_nc()
    kwargs = {}
    if _trace:
        kwargs = {"trace": True, "tmpdir": _tmpdir}
    res = run_bass_kernel_spmd(nc, in_maps, core_ids=list(range(NCORES)), **kwargs)

    out = np.empty((B, 2 * C, 1), dtype=np.float32)
    for i in range(NCORES):
        # y[b, p, k] -> channel 128k+p
        yi = res.results[i]["y"].transpose(0, 2, 1).reshape(BLOC, 2 * C)
        out[i * BLOC:(i + 1) * BLOC, :, 0] = yi
    if _trace:
        return out, res
    return out


# revision 15
# speedup vs baseline: 1.2981x; 1.1349x over previous
"""AttentiveStatsPool Trainium2 Bass kernel.

Full-input contract: kernel(**inputs) takes the unsharded numpy inputs and
returns the full (B, 2C, 1) output.  Internally shards the batch (B=16)
across 8 NeuronCores (2 samples per core), weights replicated, no cross-core
communication.

Math per sample (mask is all-ones per the problem spec):
  S1 = sum_t x, S2 = sum_t x^2                        (per channel)
  mean0 = S1/T, std0 = sqrt(max(S2/T - mean0^2, 1e-5))
  m1 = w1[:, :C] @ x            (H, T)
  cH = w1[:, C:2C] @ mean0 + w1[:, 2C:] @ std0 + b1   (H,)
  r = relu(m1 + cH)
  LN over H: h = tanh(g1 * (r - mu)*rsqrt(var+1e-5) + be1)
  z = w2 @ h                    (b2 shifts z per channel and softmax over T is
                                 shift-invariant per channel, so b2 drops out)
  u = exp(z), Z = sum_t u, M1 = sum_t u*x, M2 = sum_t u*x^2
  mean = M1/Z, std = sqrt(max(M2/Z - mean^2, 1e-5))
  out = LayerNorm_{3072}(concat(mean, std)) * g2 + be2
"""

import numpy as np
import ml_dtypes

B, C, T, H = 16, 1536, 2000, 128
NCORES = 8
BLOC = B // NCORES          # 2 samples per core
KC = C // 128               # 12 channel chunks
TH = T // 2                 # 1000, half-T psum region
TQ = T // 4                 # 500, one psum bank of f32
TSPL = 1200                 # S2 split point: [0:TSPL] on ACT, rest on DVE
EPS = 1e-5

_compiled = {}


# ---------------------------------------------------------------------------
# Workaround for walrus codegen 'Too many sync wait commands': this container's
# walrus supports only ONE sync-wait slot per instruction, but Tile's wait
# assignment can attach several.  Post-pass: move excess waits onto standalone
# InstNoOp carriers spliced immediately before the instruction on the same
# engine (same-engine program order makes this equivalent).
# ---------------------------------------------------------------------------

def _apply_tile_patch():
    import concourse.mybir as mybir
    import concourse.tile as tile
    from concourse.vector_clock import ScopedClock

    if getattr(tile.TileContext, "_wait_split_patched", False):
        return

    MAX_WAITS = 1

    def split_excess_waits(nc):
        for fn in nc.m.functions:
            for bb in fn.blocks:
                il = bb.instructions
                out = []
                changed = False
                for inst in il:
                    si = getattr(inst, "sync_info", None)
                    waits = list(si.on_wait) if si is not None else []
                    if len(waits) > MAX_WAITS:
                        for j, w in enumerate(waits[MAX_WAITS:]):
                            nop = mybir.InstNoOp(
                                name=f"{inst.name}-wsplit{j}",
                                sync_info=mybir.SyncInfo(on_wait=[w], on_update=[]),
                                bass_nofuse=True,
                                engine=inst.engine,
                            )
                            nc.register_instruction(nop, overwrite=True)
                            out.append(nop)
                        si.on_wait = waits[:MAX_WAITS]
                        changed = True
                    out.append(inst)
                if changed:
                    bb.instructions = out

    def _patched_drain_and_barrier(self, tick_clock, wait_clock):
        nc = self.nc
        drain_inst = nc.sync.drain()
        wait_clock.add_sem_waits(
            drain_inst.ins, ScopedClock({None: tick_clock.global_clock})
        )
        nc.all_engine_barrier()
        assert self.sems is not None
        popped = nc._tile_sem_poison_stack.pop()
        assert popped is self._sem_poison
        nc.clear_and_free_semaphores(list(self.sems.allocated().values()))
        nc.all_engine_barrier()
        split_excess_waits(nc)

    tile.TileContext._drain_and_barrier = _patched_drain_and_barrier
    tile.TileContext._wait_split_patched = True


# ---------------------------------------------------------------------------
# Device kernel builder (one NeuronCore, BLOC samples)
# ---------------------------------------------------------------------------

DEBUG = False


def _build():
    import concourse.bass as bass
    import concourse.tile as tile
    import concourse.mybir as mybir
    from contextlib import ExitStack

    _apply_tile_patch()

    f32 = mybir.dt.float32
    bf16 = mybir.dt.bfloat16
    AL = mybir.AluOpType
    AF = mybir.ActivationFunctionType

    nc = bass.Bass(name="attnpool")

    xd = nc.dram_tensor("x", [BLOC, KC, 128, T], f32, kind="ExternalInput")
    wad = nc.dram_tensor("wa", [128, KC, 128], bf16, kind="ExternalInput")
    wbcd = nc.dram_tensor("wbc", [128, 2 * KC, 128], f32, kind="ExternalInput")
    w2td = nc.dram_tensor("w2t", [128, KC, 128], bf16, kind="ExternalInput")
    onesd = nc.dram_tensor("ones_bf", [128, 128], bf16, kind="ExternalInput")
    onesfd = nc.dram_tensor("ones_f", [128, 128], f32, kind="ExternalInput")
    b1d = nc.dram_tensor("b1v", [128, 1], f32, kind="ExternalInput")
    g1d = nc.dram_tensor("g1v", [128, 1], f32, kind="ExternalInput")
    be1d = nc.dram_tensor("be1v", [128, 1], f32, kind="ExternalInput")
    g2d = nc.dram_tensor("g2v", [128, 2 * KC], f32, kind="ExternalInput")
    be2d = nc.dram_tensor("be2v", [128, 2 * KC], f32, kind="ExternalInput")
    yd = nc.dram_tensor("y", [BLOC, 128, 2 * KC], f32, kind="ExternalOutput")
    dbg = {}
    if DEBUG:
        for nm, shp in [("dS1", [128, 24]), ("dS2", [128, 24]), ("dmean0", [128, 24]),
                        ("dstd0", [128, 24]), ("dbias", [128, 2]), ("dh", [128, 2, 2000]),
                        ("dZ", [128, 24]), ("dM1", [128, 24]), ("dM2", [128, 24]),
                        ("dr", [128, 2, 2000])]:
            dbg[nm] = nc.dram_tensor(nm, shp, f32, kind="ExternalOutput")

    NB = BLOC * KC  # 24 accum columns, col = b*KC + k

    with tile.TileContext(nc) as tc, ExitStack() as ctx:
        singles = ctx.enter_context(tc.tile_pool(name="singles", bufs=1))
        xpool = ctx.enter_context(tc.tile_pool(name="xcache", bufs=1))
        work = ctx.enter_context(tc.tile_pool(name="work", bufs=1))
        dscr = ctx.enter_context(tc.tile_pool(name="dscr", bufs=2))

        # ---- weights / constants to SBUF ----
        wa_sb = singles.tile([128, KC, 128], bf16)
        nc.sync.dma_start(out=wa_sb, in_=wad[:, :, :])
        wbc_sb = singles.tile([128, 2 * KC, 128], f32)
        nc.sync.dma_start(out=wbc_sb, in_=wbcd[:, :, :])
        w2t_sb = singles.tile([128, KC, 128], bf16)
        nc.sync.dma_start(out=w2t_sb, in_=w2td[:, :, :])
        ones_sb = singles.tile([128, 128], bf16)
        nc.sync.dma_start(out=ones_sb, in_=onesd[:, :])
        onesf_sb = singles.tile([128, 128], f32)
        nc.sync.dma_start(out=onesf_sb, in_=onesfd[:, :])
        b1_sb = singles.tile([128, 1], f32)
        nc.sync.dma_start(out=b1_sb, in_=b1d[:, :])
        g1_sb = singles.tile([128, 1], f32)
        nc.sync.dma_start(out=g1_sb, in_=g1d[:, :])
        be1_sb = singles.tile([128, 1], f32)
        nc.sync.dma_start(out=be1_sb, in_=be1d[:, :])
        g2_sb = singles.tile([128, 2 * KC], f32)
        nc.sync.dma_start(out=g2_sb, in_=g2d[:, :])
        be2_sb = singles.tile([128, 2 * KC], f32)
        nc.sync.dma_start(out=be2_sb, in_=be2d[:, :])

        eps_sb = singles.tile([128, 1], f32)
        nc.vector.memset(eps_sb, EPS)

        # ---- persistent SBUF state ----
        x_bf = xpool.tile([128, BLOC, KC, T], bf16)          # 96 KB/part
        r_raw = work.tile([128, BLOC, T], bf16)              # m1 -> r -> t1 -> t3 -> h
        accS1 = work.tile([128, NB], f32)
        accS2a = work.tile([128, NB], f32)
        accS2b = work.tile([128, NB], f32)
        accM2a = work.tile([128, NB], f32)
        accZ = work.tile([128, NB], f32)
        accM1 = work.tile([128, NB], f32)
        accM2 = work.tile([128, NB], f32)
        mean0 = work.tile([128, NB], f32)
        std0 = work.tile([128, NB], f32)
        biasv = work.tile([128, BLOC], f32)
        mu_bf = work.tile([128, T], bf16)
        v0_f = work.tile([128, T], f32)
        rs_f = work.tile([128, T], f32)
        r2_bf = work.tile([128, T], bf16)

        # ================= pass 1: load x, m1 matmuls, S1/S2 =================
        # PSUM matmul outputs must stay inside one 2KB bank: one [128, 512]
        # f32 tile (= exactly one bank) per (sample, T-quarter), 500 used.
        with tc.tile_pool(name="pm1", bufs=1, space="PSUM") as pm1:
            m1ps = {}
            for b in range(BLOC):
                for q in range(4):
                    m1ps[(b, q)] = pm1.tile(
                        [128, 512], f32, tag=f"m1_{b}_{q}", name=f"m1_{b}_{q}"
                    )

            for k in range(KC):
                for b in range(BLOC):
                    nc.gpsimd.dma_start(out=x_bf[:, b, k, :], in_=xd[b, k, :, :])
                    for q in range(4):
                        nc.tensor.matmul(
                            m1ps[(b, q)][:, 0:TQ],
                            wa_sb[:, k, :],
                            x_bf[:, b, k, q * TQ:(q + 1) * TQ],
                            start=(k == 0),
                            stop=(k == KC - 1),
                        )
                    col = b * KC + k
                    s1scr = dscr.tile([128, T], bf16, tag="dscr")
                    nc.vector.tensor_scalar(
                        out=s1scr, in0=x_bf[:, b, k, :], scalar1=1.0, scalar2=0.0,
                        op0=AL.mult, op1=AL.add, accum_out=accS1[:, col:col + 1],
                    )
                    s2scr = dscr.tile([128, T], bf16, tag="dscr")
                    nc.scalar.activation(
                        out=s2scr, in_=x_bf[:, b, k, :],
                        func=AF.Square, accum_out=accS2a[:, col:col + 1],
                    )

            # evacuate raw m1 to SBUF (bf16)
            for b in range(BLOC):
                for q in range(4):
                    nc.scalar.copy(
                        out=r_raw[:, b, q * TQ:(q + 1) * TQ],
                        in_=m1ps[(b, q)][:, 0:TQ],
                    )

        # ---- mean0 / std0 ----
        nc.vector.tensor_scalar_mul(out=mean0, in0=accS1, scalar1=1.0 / T)
        nc.vector.tensor_scalar_mul(out=std0, in0=accS2a, scalar1=1.0 / T)
        m0sq = work.tile([128, NB], f32)
        nc.vector.scalar_tensor_tensor(
            out=m0sq, in0=mean0, scalar=1.0, in1=mean0, op0=AL.mult, op1=AL.mult
        )
        nc.vector.tensor_sub(out=std0, in0=std0, in1=m0sq)
        nc.vector.tensor_scalar_max(out=std0, in0=std0, scalar1=EPS)
        nc.scalar.activation(out=std0, in_=std0, func=AF.Ln)
        nc.scalar.activation(out=std0, in_=std0, func=AF.Exp, scale=0.5)

        # ---- cH = w1b @ mean0 + w1c @ std0 (+ b1) ----
        with tc.tile_pool(name="pmid", bufs=1, space="PSUM") as pmid:
            for b in range(BLOC):
                chps = pmid.tile([128, 1], f32, tag=f"ch{b}", name=f"ch{b}")
                for j in range(2 * KC):
                    k = j % KC
                    src = mean0 if j < KC else std0
                    nc.tensor.matmul(
                        chps,
                        wbc_sb[:, j, :],
                        src[:, b * KC + k:b * KC + k + 1],
                        start=(j == 0),
                        stop=(j == 2 * KC - 1),
                    )
                nc.vector.tensor_add(out=biasv[:, b:b + 1], in0=chps, in1=b1_sb)

        # ========== LN over H (per sample, half-T stat rounds) + pass 2 ======
        # pln holds the LN column-stat matmul outputs (2+2 banks, half T at a
        # time) and pz the z logits (4 banks), so LayerNorm of sample 1 can
        # overlap the z/exp/weighted-sum loop of sample 0.
        with tc.tile_pool(name="pln", bufs=1, space="PSUM") as pln, \
             tc.tile_pool(name="pz", bufs=1, space="PSUM") as pz:

            for b in range(BLOC):
                # r = relu(m1 + cH)  (in place on r_raw)
                nc.scalar.activation(
                    out=r_raw[:, b, :], in_=r_raw[:, b, :], func=AF.Relu,
                    bias=biasv[:, b:b + 1], scale=1.0,
                )
                if DEBUG:
                    rdump = work.tile([128, T], f32, tag="rdump")
                    nc.vector.tensor_copy(out=rdump, in_=r_raw[:, b, :])
                    nc.sync.dma_start(out=dbg["dr"][:, b, :], in_=rdump)
                nc.vector.tensor_mul(out=r2_bf, in0=r_raw[:, b, :], in1=r_raw[:, b, :])
                for hh in range(2):
                    s1b = pln.tile([128, 2, 512], f32, tag="s1b", name="s1b")
                    s2b = pln.tile([128, 2, 512], f32, tag="s2b", name="s2b")
                    for qq in range(2):
                        q = hh * 2 + qq
                        sl = slice(q * TQ, (q + 1) * TQ)
                        nc.tensor.matmul(
                            s1b[:, qq, 0:TQ], ones_sb, r_raw[:, b, sl],
                            start=True, stop=True,
                        )
                        nc.tensor.matmul(
                            s2b[:, qq, 0:TQ], ones_sb, r2_bf[:, sl],
                            start=True, stop=True,
                        )
                    hsl = slice(hh * TH, (hh + 1) * TH)
                    muv = mu_bf[:, hsl].rearrange("p (q t) -> p q t", q=2)
                    v0v = v0_f[:, hsl].rearrange("p (q t) -> p q t", q=2)
                    rsv = rs_f[:, hsl].rearrange("p (q t) -> p q t", q=2)
                    # mu = s1b/H ; var = s2b/H - mu^2 ; rs = exp(-ln(var+eps)/2)
                    nc.scalar.mul(out=muv, in_=s1b[:, :, 0:TQ], mul=1.0 / H)
                    nc.vector.scalar_tensor_tensor(
                        out=v0v, in0=muv, scalar=1.0, in1=muv,
                        op0=AL.mult, op1=AL.mult,
                    )
                    nc.vector.scalar_tensor_tensor(
                        out=v0v, in0=s2b[:, :, 0:TQ], scalar=1.0 / H, in1=v0v,
                        op0=AL.mult, op1=AL.subtract,
                    )
                    nc.vector.tensor_scalar_max(
                        out=v0_f[:, hsl], in0=v0_f[:, hsl], scalar1=0.0
                    )
                    nc.scalar.activation(
                        out=rs_f[:, hsl], in_=v0_f[:, hsl], func=AF.Ln,
                        bias=eps_sb, scale=1.0,
                    )
                    nc.scalar.activation(
                        out=v0_f[:, hsl], in_=rs_f[:, hsl], func=AF.Exp, scale=-0.5
                    )
                # h = tanh(g1*(r-mu)*rs + be1)   (in place on r_raw)
                nc.vector.tensor_sub(
                    out=r_raw[:, b, :], in0=r_raw[:, b, :], in1=mu_bf
                )
                nc.vector.tensor_mul(
                    out=r_raw[:, b, :], in0=r_raw[:, b, :], in1=v0_f
                )
                nc.scalar.activation(
                    out=r_raw[:, b, :], in_=r_raw[:, b, :], func=AF.Tanh,
                    bias=be1_sb, scale=g1_sb,
                )
                if DEBUG:
                    hdump = work.tile([128, T], f32, tag="hdump")
                    nc.vector.tensor_copy(out=hdump, in_=r_raw[:, b, :])
                    nc.sync.dma_start(out=dbg["dh"][:, b, :], in_=hdump)

                # ---- pass 2 for this sample: z, exp, weighted sums ----
                for k in range(KC):
                    zps = pz.tile([128, 4, 512], f32, tag="z", name="z")
                    for q in range(4):
                        nc.tensor.matmul(
                            zps[:, q, 0:TQ], w2t_sb[:, k, :],
                            r_raw[:, b, q * TQ:(q + 1) * TQ],
                            start=True, stop=True,
                        )
                    col = b * KC + k
                    u_bf = dscr.tile([128, T], bf16, tag="u")
                    nc.scalar.activation(
                        out=u_bf.rearrange("p (q t) -> p q t", q=4),
                        in_=zps[:, :, 0:TQ], func=AF.Exp,
                        accum_out=accZ[:, col:col + 1],
                    )
                    # p = u*x, M1 = sum(p)   (fused, DVE)
                    p_bf = dscr.tile([128, T], bf16, tag="p")
                    nc.vector.scalar_tensor_tensor(
                        out=p_bf, in0=u_bf, scalar=1.0, in1=x_bf[:, b, k, :],
                        op0=AL.mult, op1=AL.mult,
                        accum_out=accM1[:, col:col + 1],
                    )
                    # M2 split: low half fused on DVE, high half TT + ACT reduce
                    q_bf = dscr.tile([128, T], bf16, tag="q")
                    nc.vector.scalar_tensor_tensor(
                        out=q_bf[:, 0:TH], in0=p_bf[:, 0:TH], scalar=1.0,
                        in1=x_bf[:, b, k, 0:TH],
                        op0=AL.mult, op1=AL.mult,
                        accum_out=accM2[:, col:col + 1],
                    )
                    nc.vector.tensor_mul(
                        out=q_bf[:, TH:T], in0=p_bf[:, TH:T],
                        in1=x_bf[:, b, k, TH:T],
                    )
                    qs_bf = dscr.tile([128, TH], bf16, tag="qs")
                    nc.scalar.activation(
                        out=qs_bf, in_=q_bf[:, TH:T], func=AF.Copy,
                        accum_out=accM2a[:, col:col + 1],
                    )

        # ================= final stats + LayerNorm(3072) =================
        if DEBUG:
            nc.sync.dma_start(out=dbg["dS1"][:, :], in_=accS1)
            nc.sync.dma_start(out=dbg["dS2"][:, :], in_=accS2a)
            nc.sync.dma_start(out=dbg["dmean0"][:, :], in_=mean0)
            nc.sync.dma_start(out=dbg["dstd0"][:, :], in_=std0)
            nc.sync.dma_start(out=dbg["dbias"][:, :], in_=biasv)
            nc.sync.dma_start(out=dbg["dZ"][:, :], in_=accZ)
            nc.sync.dma_start(out=dbg["dM1"][:, :], in_=accM1)
            nc.sync.dma_start(out=dbg["dM2"][:, :], in_=accM2)
        zr = work.tile([128, NB], f32)
        nc.vector.reciprocal(out=zr, in_=accZ)
        vmean = work.tile([128, NB], f32)
        nc.vector.tensor_mul(out=vmean, in0=accM1, in1=zr)
        ve2 = work.tile([128, NB], f32)
        nc.vector.tensor_add(out=ve2, in0=accM2, in1=accM2a)
        nc.vector.tensor_mul(out=ve2, in0=ve2, in1=zr)
        vmsq = work.tile([128, NB], f32)
        nc.vector.scalar_tensor_tensor(
            out=vmsq, in0=vmean, scalar=1.0, in1=vmean, op0=AL.mult, op1=AL.mult
        )
        nc.vector.tensor_sub(out=ve2, in0=ve2, in1=vmsq)
        nc.vector.tensor_scalar_max(out=ve2, in0=ve2, scalar1=EPS)
        nc.scalar.activation(out=ve2, in_=ve2, func=AF.Ln)
        nc.scalar.activation(out=ve2, in_=ve2, func=AF.Exp, scale=0.5)  # std

        with tc.tile_pool(name="pfin", bufs=1, space="PSUM") as pf:
            for b in range(BLOC):
                v = work.tile([128, 2 * KC], f32, tag="vfin")
                nc.vector.tensor_copy(out=v[:, 0:KC], in_=vmean[:, b * KC:(b + 1) * KC])
                nc.vector.tensor_copy(out=v[:, KC:2 * KC], in_=ve2[:, b * KC:(b + 1) * KC])
                v2 = work.tile([128, 2 * KC], f32, tag="v2fin")
                nc.scalar.square(out=v2, in_=v)
                svp = pf.tile([128, 2 * KC], f32, tag="sv")
                nc.tensor.matmul(svp, onesf_sb, v, start=True, stop=True)
                sv2p = pf.tile([128, 2 * KC], f32, tag="sv2")
                nc.tensor.matmul(sv2p, onesf_sb, v2, start=True, stop=True)
                muf = work.tile([128, 1], f32, tag="muf")
                nc.vector.tensor_reduce(
                    out=muf, in_=svp, axis=mybir.AxisListType.X, op=AL.add
                )
                s2r = work.tile([128, 1], f32, tag="s2r")
                nc.vector.tensor_reduce(
                    out=s2r, in_=sv2p, axis=mybir.AxisListType.X, op=AL.add
                )
                nc.vector.tensor_scalar_mul(out=muf, in0=muf, scalar1=1.0 / (2 * C))
                musq = work.tile([128, 1], f32, tag="musq")
                nc.vector.scalar_tensor_tensor(
                    out=musq, in0=muf, scalar=1.0, in1=muf, op0=AL.mult, op1=AL.mult
                )
                nc.vector.scalar_tensor_tensor(
                    out=s2r, in0=s2r, scalar=1.0 / (2 * C), in1=musq,
                    op0=AL.mult, op1=AL.subtract,
                )
                nc.scalar.activation(
                    out=s2r, in_=s2r, func=AF.Ln, bias=eps_sb, scale=1.0
                )
                nc.scalar.activation(out=s2r, in_=s2r, func=AF.Exp, scale=-0.5)
                vout = work.tile([128, 2 * KC], f32, tag="vout")
                nc.vector.tensor_scalar(
                    out=vout, in0=v, scalar1=muf, scalar2=s2r,
                    op0=AL.subtract, op1=AL.mult,
                )
                nc.vector.tensor_mul(out=vout, in0=vout, in1=g2_sb)
                nc.vector.tensor_add(out=vout, in0=vout, in1=be2_sb)
                nc.sync.dma_start(out=yd[b, :, :], in_=vout)

    return nc


def _get_nc():
    if "nc" not in _compiled:
        _compiled["nc"] = _build()
    return _compiled["nc"]


def _prep_common(w1, b1, g1, be1, w2, g2, be2):
    bf = ml_dtypes.bfloat16
    # SBUF-layout weights (partition-major, contiguous DMA):
    # wa[c, k, h] = w1[h, 128k+c] ; wbc[c, j, h] ; w2t[h, k, c] = w2[128k+c, h]
    w1 = np.asarray(w1, np.float32)
    w1a = np.ascontiguousarray(
        w1[:, :C].T.reshape(KC, 128, H).transpose(1, 0, 2)).astype(bf)
    w1bT = w1[:, C:2 * C].T.reshape(KC, 128, H)
    w1cT = w1[:, 2 * C:].T.reshape(KC, 128, H)
    wbc = np.ascontiguousarray(
        np.concatenate([w1bT, w1cT], axis=0).transpose(1, 0, 2)
    ).astype(np.float32)
    w2t = np.ascontiguousarray(
        np.asarray(w2, np.float32).reshape(KC, 128, H).transpose(2, 0, 1)
    ).astype(bf)

    return {
        "wa": w1a,
        "wbc": wbc,
        "w2t": w2t,
        "ones_bf": np.ones((128, 128), dtype=bf),
        "ones_f": np.ones((128, 128), dtype=np.float32),
        "b1v": np.asarray(b1, np.float32).reshape(128, 1),
        "g1v": np.asarray(g1, np.float32).reshape(128, 1),
        "be1v": np.asarray(be1, np.float32).reshape(128, 1),
        "g2v": np.ascontiguousarray(np.asarray(g2, np.float32).reshape(2 * KC, 128).T),
        "be2v": np.ascontiguousarray(np.asarray(be2, np.float32).reshape(2 * KC, 128).T),
    }


def kernel(x, mask, w1, b1, g1, be1, w2, b2, g2, be2, _trace=False, _tmpdir=None):
    from concourse.bass_utils import run_bass_kernel_spmd

    x = np.asarray(x, dtype=np.float32)
    common = _prep_common(w1, b1, g1, be1, w2, g2, be2)

    in_maps = []
    for i in range(NCORES):
        xi = np.ascontiguousarray(
            x[i * BLOC:(i + 1) * BLOC].reshape(BLOC, KC, 128, T)
        )
        in_maps.append({"x": xi, **common})

    nc = _get_nc()
    kwargs = {}
    if _trace:
        kwargs = {"trace": True, "tmpdir": _tmpdir}
    res = run_bass_kernel_spmd(nc, in_maps, core_ids=list(range(NCORES)), **kwargs)

    out = np.empty((B, 2 * C, 1), dtype=np.float32)
    for i in range(NCORES):
        # y[b, p, k] -> channel 128k+p
        yi = res.results[i]["y"].transpose(0, 2, 1).reshape(BLOC, 2 * C)
        out[i * BLOC:(i + 1) * BLOC, :, 0] = yi
    if _trace:
        return out, res
    return out
